# revision 27
# baseline (speedup 1.0000x reference)
"""KAN-attention kernel for 8 Trainium2 NeuronCores.

Math: y[b,o] = sum_i [ sb[o,i]*silu(x[b,i]) + sp[o,i]*sum_c B[b,i,c]*coef[o,i,c] ]
for the q and k branches summed, + bias, softmax over groups of out_dim=8.

Strategy: shard act_out (o = 2048) into 8 contiguous slices of 256 (tensor
parallel). Softmax groups (8) never straddle a slice -> zero collectives.

Fast path (masks and scales all ones, which is what setup_inputs produces):
sb = sp = 1, so the base term sum_i silu(x[b,i]) is the same for every o and
cancels exactly in the softmax over out_dim -> drop it. What remains is a pure
contraction y[b,o] = sum_{i,c} B[b,i,c]*coef[o,i,c] streamed through the PE
with no on-device elementwise work at all. coef is host-transposed to
(cpair, ip, c2, iblk, o) fp16: halves HBM traffic (the only real cost of this
kernel) and keeps every DMA at 16KB/partition contiguous. fp16 keeps 10
mantissa bits; measured end-to-end rel err ~6e-4 vs the 2e-2 gate.

General path (any mask/scale not 1): the original fp32r kernel with on-device
mask*scale multiplies, kept verbatim as fallback.
"""

import numpy as np

NUM_F, POLY = 5, 3
C = NUM_F + POLY          # 8 spline coefficients per edge
BATCH, HEADS, PATCHES, DIM, OUT_DIM = 8, 4, 64, 8, 8
ACT_IN = HEADS * PATCHES * DIM      # 2048
ACT_OUT = HEADS * PATCHES * OUT_DIM  # 2048
N_CORES = 8
O_PER = ACT_OUT // N_CORES          # 256
IBLK, IP = 16, 128                  # i = iblk*128 + ip
NGRP = O_PER // OUT_DIM             # 32 softmax groups per core


def _ext_grid():
    g = np.linspace(-1.0, 1.0, NUM_F + 1)
    h = 2.0 / NUM_F
    left = g[0] - h * np.arange(POLY, 0, -1)
    right = g[-1] + h * np.arange(1, POLY + 1)
    return np.concatenate([left, g, right]).astype(np.float32)  # (12,)


def _bspline_basis(x, grid, k):
    # Cox-de Boor; x: (..., ), grid: (G,) -> (..., G-1-k)
    x1 = x[..., None]
    B = ((x1 >= grid[:-1]) & (x1 < grid[1:])).astype(x.dtype)
    for d in range(1, k + 1):
        left = (x1 - grid[:-(d + 1)]) / (grid[d:-1] - grid[:-(d + 1)])
        right = (grid[d + 1:] - x1) / (grid[d + 1:] - grid[1:-d])
        B = left * B[..., :-1] + right * B[..., 1:]
    return B


_NC_FAST = []
_NC_GENERAL = []


def _build_fast():
    """Pure fp16 coef-stream program: 257 matmuls, no elementwise prologue."""
    if _NC_FAST:
        return _NC_FAST[0]
    from contextlib import ExitStack

    import concourse.bacc as bacc
    import concourse.tile as tile
    from concourse import mybir

    dt = mybir.dt
    f32 = dt.float32
    f16 = dt.float16

    nc = bacc.Bacc(target_bir_lowering=False, trn_type="TRN2", debug=False)

    NCH = C // 2  # 4 channel-pair chunks per branch
    cq = nc.dram_tensor("cq", [NCH, IP, 2, IBLK, O_PER], f16,
                        kind="ExternalInput").ap()
    ck = nc.dram_tensor("ck", [NCH, IP, 2, IBLK, O_PER], f16,
                        kind="ExternalInput").ap()
    fq = nc.dram_tensor("fq", [IP, C, IBLK, BATCH], f16, kind="ExternalInput").ap()
    fk = nc.dram_tensor("fk", [IP, C, IBLK, BATCH], f16, kind="ExternalInput").ap()
    ones = nc.dram_tensor("ones", [1, BATCH], f32, kind="ExternalInput").ap()
    biasr = nc.dram_tensor("biasr", [1, O_PER], f32, kind="ExternalInput").ap()
    yout = nc.dram_tensor("yout", [BATCH, O_PER], f32, kind="ExternalOutput").ap()

    with tile.TileContext(nc) as tc, ExitStack() as ctx:
        feats = ctx.enter_context(tc.tile_pool(name="feats", bufs=1))
        coefs = ctx.enter_context(tc.tile_pool(name="coefs", bufs=1))
        outp = ctx.enter_context(tc.tile_pool(name="outp", bufs=1))
        psum = ctx.enter_context(tc.tile_pool(name="psum", bufs=1, space="PSUM"))

        # Everything the PE needs early (both feature tensors) loads FIRST
        # on the same sync HWDGE ring that carries the coef stream: the ring
        # is FIFO, so they are guaranteed in SBUF before chunk 0 finishes.
        # The scalar ring only carries the two tiny bias operands, which are
        # not needed until the very last matmul -- a starved scalar ring
        # (big sync packets monopolize the SDMA round-robin) costs nothing.
        fq_sb = feats.tile([IP, C, IBLK, BATCH], f16)
        fk_sb = feats.tile([IP, C, IBLK, BATCH], f16)
        ones_sb = feats.tile([1, BATCH], f32)
        biasr_sb = feats.tile([1, O_PER], f32)
        nc.sync.dma_start(out=fq_sb[:], in_=fq[:])
        nc.sync.dma_start(out=fk_sb[:], in_=fk[:])
        nc.scalar.dma_start(out=ones_sb[:], in_=ones[:])
        nc.scalar.dma_start(out=biasr_sb[:], in_=biasr[:])

        # The whole coef stream rides the sync ring, one full 128-partition
        # transfer per 2MB channel-pair chunk, in PE consumption order: ring
        # FIFO guarantees chunks complete exactly in the order the PE needs
        # them at the full 16-engine rate. (Chunk-per-ring alternation
        # couples the PE to run-to-run ring arbitration: measured 10us+
        # stalls. Half-partition transfers halve SDMA packet size and with
        # it bandwidth. 4MB chunks make packets so big the other ring
        # starves for 30us.) The stream tapers (1MB/0.5MB/0.5MB) at the end
        # so the PE drain after the last byte lands is 8 matmuls. Each
        # chunk has its own buffer: all DMAs are in flight from the start.
        # The PE's very last dependency is pre-staged on the scalar ring at
        # t=0: the final channel's last 8 iblks (0.5MB) trickle in behind
        # the tiny bias operands with ~40us of slack, so when the sync
        # stream's last byte lands the PE finishes on already-resident data.
        chunks = []  # (tile, f_sb, [(tile_cc, c, ib_lo, ib_hi), ...])
        pre = []
        for bi, (c_dram, f_sb) in enumerate(((cq, fq_sb), (ck, fk_sb))):
            for ch in range(NCH):
                c0 = 2 * ch
                if bi == 1 and ch == NCH - 1:
                    cbb = coefs.tile([IP, IBLK, O_PER], f16, tag="coefb")
                    nc.sync.dma_start(out=cbb[:], in_=c_dram[ch, :, 0])
                    chunks.append((cbb, f_sb, [(None, c0, 0, IBLK)]))
                    cbc = coefs.tile([IP, 8, O_PER], f16, tag="coefc")
                    nc.sync.dma_start(out=cbc[:], in_=c_dram[ch, :, 1, 0:8])
                    chunks.append((cbc, f_sb, [(None, c0 + 1, 0, 8)]))
                    for pi, (lo, hi) in enumerate(((8, 12), (12, 16))):
                        cbp = coefs.tile([IP, hi - lo, O_PER], f16,
                                         tag=f"coefp{pi}")
                        nc.scalar.dma_start(out=cbp[:],
                                            in_=c_dram[ch, :, 1, lo:hi])
                        pre.append((cbp, f_sb, [(None, c0 + 1, lo, hi)]))
                else:
                    cb = coefs.tile([IP, 2, IBLK, O_PER], f16,
                                    tag=f"coef{bi}{ch}")
                    nc.sync.dma_start(out=cb[:], in_=c_dram[ch])
                    chunks.append((cb, f_sb,
                                   [(cc, c0 + cc, 0, IBLK) for cc in range(2)]))
        chunks += pre

        # matmul 0 folds the bias into the accumulation group (ones[1,8]^T
        # @ bias[1,256] broadcasts bias over the batch rows); its operands
        # arrive on the scalar ring long before chunk 0 does on sync
        Y = psum.tile([BATCH, O_PER], f32)
        nc.tensor.matmul(Y[:], ones_sb[:], biasr_sb[:], start=True, stop=False)

        n_mm = 2 * C * IBLK  # 256 coef matmuls after the bias matmul
        mm = 0
        for cb, f_sb, parts in chunks:
            for cc, c, ib_lo, ib_hi in parts:
                for ib in range(ib_lo, ib_hi):
                    mov = cb[:, ib - ib_lo, :] if cc is None else cb[:, cc, ib, :]
                    nc.tensor.matmul(
                        Y[:], f_sb[:, c, ib, :], mov,
                        start=False, stop=(mm == n_mm - 1),
                    )
                    mm += 1

        # softmax over groups of 8 along the free dim, no transpose needed:
        # logits are bounded (|y| < ~15) so exp() without max-shift is safe
        e = outp.tile([BATCH, NGRP, OUT_DIM], f32)
        nc.scalar.activation(
            e[:], Y[:].rearrange("b (g d) -> b g d", d=OUT_DIM),
            mybir.ActivationFunctionType.Exp,
        )
        sm = outp.tile([BATCH, NGRP], f32)
        nc.vector.tensor_reduce(
            sm[:], e[:], axis=mybir.AxisListType.X, op=mybir.AluOpType.add,
        )
        nc.vector.reciprocal(sm[:], sm[:])
        yt = outp.tile([BATCH, O_PER], f32)
        nc.vector.tensor_tensor(
            yt[:].rearrange("b (g d) -> b g d", d=OUT_DIM), e[:],
            sm[:].unsqueeze(2).broadcast_to((BATCH, NGRP, OUT_DIM)),
            mybir.AluOpType.mult,
        )
        nc.sync.dma_start(out=yout[:], in_=yt[:])

    nc.compile()
    _NC_FAST.append(nc)
    return nc


def _host_prep_fast(q, k, coef_q, coef_k, bias_w):
    grid = _ext_grid()
    xq = np.ascontiguousarray(q, np.float32).reshape(BATCH, ACT_IN)
    xk = np.ascontiguousarray(k, np.float32).reshape(BATCH, ACT_IN)

    def feat(x):
        B = _bspline_basis(x, grid, POLY)                      # (8, 2048, 8)
        fb = B.reshape(BATCH, IBLK, IP, C).transpose(2, 3, 1, 0)  # (128,8,16,8)
        return np.ascontiguousarray(fb, np.float16)

    fq_h, fk_h = feat(xq), feat(xk)

    def cslices(t):
        # (N, C) edges n = o*2048+i, i = ib*128+ip -> per-core fp16
        # [C//2, IP, 2, IBLK, O_PER] (16KB contiguous per partition per chunk)
        t16 = np.asarray(t).astype(np.float16)
        v = t16.reshape(N_CORES, O_PER, IBLK, IP, C // 2, 2)
        v = v.transpose(0, 4, 3, 5, 2, 1)
        return np.ascontiguousarray(v)

    cq_h, ck_h = cslices(coef_q), cslices(coef_k)
    bias_h = np.asarray(bias_w, np.float32).reshape(N_CORES, 1, O_PER)
    ones_h = np.ones((1, BATCH), np.float32)

    in_maps = []
    for m in range(N_CORES):
        in_maps.append({
            "cq": cq_h[m], "ck": ck_h[m],
            "fq": fq_h, "fk": fk_h,
            "ones": ones_h,
            "biasr": np.ascontiguousarray(bias_h[m]),
        })
    return in_maps


# ---------------------------------------------------------------------------
# general fallback: original fp32r kernel with on-device mask*scale streams
# ---------------------------------------------------------------------------

def _build_general():
    if _NC_GENERAL:
        return _NC_GENERAL[0]
    from contextlib import ExitStack

    import concourse.bacc as bacc
    import concourse.tile as tile
    from concourse import mybir

    dt = mybir.dt
    f32 = dt.float32
    f32r = dt.float32r

    nc = bacc.Bacc(target_bir_lowering=False, trn_type="TRN2", debug=False)

    cq = nc.dram_tensor("cq", [C, IP, IBLK, O_PER], f32, kind="ExternalInput").ap()
    ck = nc.dram_tensor("ck", [C, IP, IBLK, O_PER], f32, kind="ExternalInput").ap()
    mq = nc.dram_tensor("mq", [IP, IBLK, O_PER], f32, kind="ExternalInput").ap()
    mk = nc.dram_tensor("mk", [IP, IBLK, O_PER], f32, kind="ExternalInput").ap()
    ssp = nc.dram_tensor("ssp", [IP, IBLK, O_PER], f32, kind="ExternalInput").ap()
    sbs = nc.dram_tensor("sbs", [IP, IBLK, O_PER], f32, kind="ExternalInput").ap()
    fq = nc.dram_tensor("fq", [IP, C + 1, IBLK, BATCH], f32, kind="ExternalInput").ap()
    fk = nc.dram_tensor("fk", [IP, C + 1, IBLK, BATCH], f32, kind="ExternalInput").ap()
    bias = nc.dram_tensor("bias", [BATCH, O_PER], f32, kind="ExternalInput").ap()
    yout = nc.dram_tensor("yout", [IP, 2, OUT_DIM], f32, kind="ExternalOutput").ap()

    with tile.TileContext(nc) as tc, ExitStack() as ctx:
        feats = ctx.enter_context(tc.tile_pool(name="feats", bufs=1))
        scales = ctx.enter_context(tc.tile_pool(name="scales", bufs=1))
        masks = ctx.enter_context(tc.tile_pool(name="masks", bufs=1))
        sps = ctx.enter_context(tc.tile_pool(name="sps", bufs=2))
        coefs = ctx.enter_context(tc.tile_pool(name="coefs", bufs=3))
        ws = ctx.enter_context(tc.tile_pool(name="ws", bufs=3))
        outp = ctx.enter_context(tc.tile_pool(name="outp", bufs=1))
        psum = ctx.enter_context(tc.tile_pool(name="psum", bufs=1, space="PSUM"))

        # minimal prefix for the spline stream: mask_q + scale_sp + fq only
        ssp_sb = scales.tile([IP, IBLK, O_PER], f32)
        nc.sync.dma_start(out=ssp_sb[:], in_=ssp[:])
        fq_sb = feats.tile([IP, C + 1, IBLK, BATCH], f32)
        fk_sb = feats.tile([IP, C + 1, IBLK, BATCH], f32)
        nc.sync.dma_start(out=fq_sb[:], in_=fq[:])
        # float32r copies of the spline features (fp32r matmul: ~2 cyc/row
        # warm, near-fp32r accuracy; bf16 was 11x worse on rel err)
        fqr_sb = feats.tile([IP, C, IBLK, BATCH], f32r)
        fkr_sb = feats.tile([IP, C, IBLK, BATCH], f32r)
        nc.vector.tensor_copy(out=fqr_sb[:], in_=fq_sb[:, :C])

        sbs_sb = scales.tile([IP, IBLK, O_PER], f32)
        bias_sb = outp.tile([BATCH, O_PER], f32)

        Y = psum.tile([BATCH, O_PER], f32)

        n_mm = 2 * (IBLK + C * IBLK)  # 288
        mm = [0]

        def flags():
            i = mm[0]
            mm[0] += 1
            return dict(start=(i == 0), stop=(i == n_mm - 1))

        SPL = 11  # iblk split: DVE does [0:11], GpSimd [11:16] (~2x slower/elem)
        for bi, (m_dram, c_dram, f_sb, fr_sb) in enumerate(
            ((mq, cq, fq_sb, fqr_sb), (mk, ck, fk_sb, fkr_sb))
        ):
            m_sb = masks.tile([IP, IBLK, O_PER], f32, tag="mask")
            nc.sync.dma_start(out=m_sb[:], in_=m_dram[:])

            # spline weights sp = mask * scale_sp, reused by all 8 c-blocks
            sp_sb = sps.tile([IP, IBLK, O_PER], f32, tag="sp")
            nc.vector.tensor_mul(sp_sb[:], m_sb[:], ssp_sb[:])

            for c in range(C):
                cb = coefs.tile([IP, IBLK, O_PER], f32, tag="coef")
                nc.sync.dma_start(out=cb[:], in_=c_dram[c])
                w = ws.tile([IP, IBLK, O_PER], f32r, tag="w")
                # split each block's elementwise stream across DVE and GpSimd
                nc.vector.tensor_tensor(
                    w[:, :SPL], cb[:, :SPL], sp_sb[:, :SPL], mybir.AluOpType.mult)
                nc.gpsimd.tensor_tensor(
                    w[:, SPL:], cb[:, SPL:], sp_sb[:, SPL:], mybir.AluOpType.mult)
                for ib in range(IBLK):
                    nc.tensor.matmul(
                        Y[:], fr_sb[:, c, ib, :], w[:, ib, :], **flags(),
                    )

            # base term: weights sb = mask * scale_base, plain fp32 matmul
            if bi == 0:
                nc.sync.dma_start(out=sbs_sb[:], in_=sbs[:])
            sb_w = ws.tile([IP, IBLK, O_PER], f32, tag="w")
            nc.vector.tensor_mul(sb_w[:], m_sb[:], sbs_sb[:])
            for ib in range(IBLK):
                nc.tensor.matmul(Y[:], f_sb[:, C, ib, :], sb_w[:, ib, :], **flags())

            # stage the k branch's inputs behind the q branch's stream
            if bi == 0:
                nc.sync.dma_start(out=fk_sb[:], in_=fk[:])
                nc.vector.tensor_copy(out=fkr_sb[:], in_=fk_sb[:, :C])
                nc.sync.dma_start(out=bias_sb[:], in_=bias[:])

        # y = Y + bias, then regroup to [(b%4)*32+g, b//4, d] for the softmax
        ybuf = outp.tile([BATCH, O_PER], f32)
        nc.vector.tensor_add(ybuf[:], Y[:], bias_sb[:])

        yt = outp.tile([IP, 2, OUT_DIM], f32)
        for h in range(2):
            src = ybuf[4 * h:4 * h + 4, :].rearrange("b (g d) -> b g d", d=OUT_DIM)
            nc.sync.dma_start(out=yt[:, h, :], in_=src)

        # softmax over d within each partition row: exp(y - max) / sum
        mx = outp.tile([IP, 2], f32)
        sm = outp.tile([IP, 2], f32)
        for h in range(2):
            nc.vector.tensor_reduce(
                mx[:, h:h + 1], yt[:, h, :],
                axis=mybir.AxisListType.X, op=mybir.AluOpType.max, negate=True,
            )
            nc.scalar.activation(
                yt[:, h, :], yt[:, h, :],
                mybir.ActivationFunctionType.Exp, bias=mx[:, h:h + 1],
            )
            nc.vector.tensor_reduce(
                sm[:, h:h + 1], yt[:, h, :],
                axis=mybir.AxisListType.X, op=mybir.AluOpType.add,
            )
        nc.vector.reciprocal(sm[:], sm[:])
        for h in range(2):
            nc.vector.tensor_scalar_mul(yt[:, h, :], yt[:, h, :], sm[:, h:h + 1])

        nc.sync.dma_start(out=yout[:], in_=yt[:])

    nc.compile()
    _NC_GENERAL.append(nc)
    return nc


def _host_prep_general(q, k, coef_q, coef_k, scale_base, scale_sp,
                       mask_q, mask_k, bias_w):
    grid = _ext_grid()
    xq = np.ascontiguousarray(q, np.float32).reshape(BATCH, ACT_IN)
    xk = np.ascontiguousarray(k, np.float32).reshape(BATCH, ACT_IN)

    def feat(x):
        B = _bspline_basis(x, grid, POLY)            # (8, 2048, 8)
        silu = (x / (1.0 + np.exp(-x))).astype(np.float32)
        fb = B.reshape(BATCH, IBLK, IP, C).transpose(2, 3, 1, 0)   # (128,8,16,8)
        fs = silu.reshape(BATCH, IBLK, IP).transpose(2, 1, 0)      # (128,16,8)
        return np.ascontiguousarray(
            np.concatenate([fb, fs[:, None]], axis=1), np.float32)  # (128,9,16,8)

    fq_h, fk_h = feat(xq), feat(xk)

    def wslices(t):  # (N,...) over edges n = o*2048+i -> per-core (.., 128, 16, 256)
        t = np.asarray(t, np.float32)
        if t.ndim == 1:  # scale/mask: (N,) -> (m, 128, 16, 256)
            v = t.reshape(N_CORES, O_PER, IBLK, IP).transpose(0, 3, 2, 1)
        else:            # coef: (N, C) -> (m, C, 128, 16, 256)
            v = t.reshape(N_CORES, O_PER, IBLK, IP, C).transpose(0, 4, 3, 2, 1)
        return np.ascontiguousarray(v)

    cq_h, ck_h = wslices(coef_q), wslices(coef_k)
    mq_h, mk_h = wslices(mask_q), wslices(mask_k)
    ssp_h, sbs_h = wslices(scale_sp), wslices(scale_base)
    bias_h = np.asarray(bias_w, np.float32).reshape(N_CORES, 1, O_PER)

    in_maps = []
    for m in range(N_CORES):
        in_maps.append({
            "cq": cq_h[m], "ck": ck_h[m],
            "mq": mq_h[m], "mk": mk_h[m],
            "ssp": ssp_h[m], "sbs": sbs_h[m],
            "fq": fq_h, "fk": fk_h,
            "bias": np.ascontiguousarray(np.broadcast_to(bias_h[m], (BATCH, O_PER))),
        })
    return in_maps


def _assemble_fast(results):
    # yout [8, 256] = the (batch, o_slice) block of the flat output
    out = np.empty((BATCH, HEADS, PATCHES, OUT_DIM), np.float32)
    flat = out.reshape(BATCH, ACT_OUT)
    for m, r in enumerate(results):
        flat[:, m * O_PER:(m + 1) * O_PER] = r["yout"]
    return out


def _assemble(results):
    # yout [128, 2, 8]: partition p = b_lo*32 + g, free = (b_hi, d); b = b_hi*4+b_lo
    out = np.empty((BATCH, HEADS, PATCHES, OUT_DIM), np.float32)
    flat = out.reshape(BATCH, ACT_OUT)
    for m, r in enumerate(results):
        y = r["yout"].reshape(4, 32, 2, OUT_DIM)          # (b_lo, g, b_hi, d)
        y = y.transpose(2, 0, 1, 3).reshape(BATCH, O_PER)  # (b, g*8+d)
        flat[:, m * O_PER:(m + 1) * O_PER] = y
    return out


def _all_ones(a):
    a = np.asarray(a)
    return a.dtype.kind == 'f' and a.min() == 1.0 and a.max() == 1.0


def kernel(q, k, coef_q, coef_k, scale_base, scale_sp, mask_q, mask_k, bias_w,
           _trace=False):
    from concourse.bass_utils import run_bass_kernel_spmd

    fast = all(_all_ones(a) for a in (scale_base, scale_sp, mask_q, mask_k))
    if fast:
        nc = _build_fast()
        in_maps = _host_prep_fast(q, k, coef_q, coef_k, bias_w)
    else:
        nc = _build_general()
        in_maps = _host_prep_general(q, k, coef_q, coef_k, scale_base, scale_sp,
                                     mask_q, mask_k, bias_w)
    res = run_bass_kernel_spmd(nc, in_maps, core_ids=list(range(N_CORES)),
                               trace=_trace)
    out = _assemble_fast(res.results) if fast else _assemble(res.results)
    if _trace:
        return out, res
    return out
